# revision 21
# baseline (speedup 1.0000x reference)
"""Multi-head causal attention (RoPE) forward on 8 Trainium2 NeuronCores.

Sharding: tensor-parallel over heads -- 8 cores x 2 heads, each core handling
both batch elements (the flattened (B*T) = 4096 "time" axis).
Per core:
  phase 1: qT/kT [d, B*T] and v [B*T, d] projections from host-pre-transposed
           xT, RoPE applied via a +-1 pair-swap permutation matmul on PE plus
           elementwise combine with host-precomputed interleaved cos/sin.
  phase 2: per (head, batch), scores^T [j, i] = kT^T @ qT, exp on ScalarE (no
           max pass -- the score distribution is bounded and softmax is
           shift-invariant), mask as additive bias on partially-masked tiles
           only, fully-masked tiles skipped; out^T [d, i] and the softmax
           denominators (ones-matmul) accumulate on PE; normalized on evac.
           Each head's attention output leaves via its own 8-rank AllToAll
           (head-split -> t-split) so comm overlaps the next head's compute.
  phase 3: y[t-slice, :] = outT_full^T @ wo with wo streamed per half-chunk.
Host assembles the 8 t-slices into the full (B, T, C) output.

All matmuls run in float32r (fp32 with 12-bit mantissa, 1 col/cycle on the
PE): measured rel err ~2e-4 vs fp64 on 2048-deep dots.
"""

import os
import sys

import numpy as np

for _p in ("/opt/trn_rl_repo", "/root/.axon_site/_ro/trn_rl_repo"):
    if os.path.isdir(_p) and _p not in sys.path:
        sys.path.append(_p)

import concourse.bacc as bacc
import concourse.tile as tile
from concourse import mybir
from concourse.bass_utils import run_bass_kernel_spmd

B, T, C = 2, 2048, 2048
N_HEADS, D = 16, 128
THETA = 10000.0
N_CORES = 8
HPC = N_HEADS // N_CORES     # heads per core
BT = B * T                   # flattened time axis
TSL = BT // N_CORES          # per-core output slice after the all-to-all
KT = C // 128                # contraction chunks
TC1 = 256                    # phase-1 t-chunk (moving free dim)
NTC1 = BT // TC1
TC2 = 512                    # phase-2/3 chunk
CI = T // TC2                # i-chunks per (head, batch)
JT = T // 128                # j-tiles per (head, batch)
SCALE = 1.0 / np.sqrt(D)
MASKED_BIAS = -1.0e6         # pre-scale units; exp(SCALE*(s+bias)) == 0

F32R = mybir.dt.float32r
F32 = mybir.dt.float32


def _round_f32r(a):
    a = np.ascontiguousarray(a, dtype=np.float32)
    return (a.view(np.uint32) & np.uint32(0xFFFFF800)).view(np.float32)


def _mask_plan(mask2d):
    """Per (ci, jt) code: None=skip (all masked), -1=free (none masked),
    >=0 = index of partial-mask bias tile. scoresT tile (jt, ci) holds
    mask2d[i, j] transposed: bias[j_loc, i_loc] <- mask2d[TC2*ci+i, 128*jt+j].
    """
    uniq = {}
    tiles = []
    plan = []
    for ci in range(CI):
        row = []
        for jt in range(JT):
            blk = mask2d[TC2 * ci:TC2 * (ci + 1), 128 * jt:128 * (jt + 1)]
            if blk.all():
                row.append(-1)
            elif not blk.any():
                row.append(None)
            else:
                bias = np.where(blk.T, 0.0, np.float32(MASKED_BIAS)).astype(np.float32)
                key = bias.tobytes()
                if key not in uniq:
                    uniq[key] = len(tiles)
                    tiles.append(bias)
                row.append(uniq[key])
        plan.append(row)
    if not tiles:  # keep the DRAM tensor non-empty
        tiles.append(np.zeros((128, TC2), np.float32))
    return plan, np.stack(tiles)


def _rope_tables():
    inv_freq = 1.0 / (THETA ** (np.arange(0, D, 2, dtype=np.float64) / D))
    freqs = np.outer(inv_freq, np.arange(T, dtype=np.float64))  # [64, T]
    cosI = np.repeat(np.cos(freqs), 2, axis=0).astype(np.float32)  # [128, T]
    sinI = np.repeat(np.sin(freqs), 2, axis=0).astype(np.float32)
    # rot = psignT.T @ x : rot[2i] = -x[2i+1], rot[2i+1] = x[2i]
    psignT = np.zeros((D, D), np.float32)
    for i in range(D // 2):
        psignT[2 * i + 1, 2 * i] = -1.0
        psignT[2 * i, 2 * i + 1] = 1.0
    return cosI, sinI, psignT


def _phase1(nc, tc, qkv_tensors, xT_r, cos_sb, sin_sb):
    qT, kT, vt, wq_h, wk_h, wv_sb, psg_sb = qkv_tensors  # w/cos/sin live in wpool
    with tc.tile_pool(name="xt", bufs=2) as xp, \
         tc.tile_pool(name="p1t", bufs=1) as p1, \
         tc.tile_pool(name="ps1", bufs=1, space="PSUM") as pp:
        for tcn in range(NTC1):
            ts = tcn * TC1           # position in flattened BT
            tp = ts % T              # rope position (restarts per batch)
            xt = xp.tile([128, KT, TC1], F32R, tag="xt")
            nparts = 4 if tcn == 0 else 2
            step = KT // nparts
            for q_ in range(nparts):
                nc.sync.dma_start(xt[:, q_ * step:(q_ + 1) * step, :],
                                  xT_r[:, q_ * step:(q_ + 1) * step,
                                       ts:ts + TC1])
            for dst, w_h in ((qT, wq_h), (kT, wk_h)):
                for h in range(HPC):
                    ps = pp.tile([D, TC1], F32, tag="proj", bufs=4)
                    for cc in range(KT):
                        nc.tensor.matmul(
                            ps[:], w_h[h][:, cc, :], xt[:, cc, :],
                            start=(cc == 0), stop=(cc == KT - 1))
                    praw = p1.tile([D, TC1], F32R, tag="praw", bufs=3)
                    nc.vector.tensor_copy(praw[:], ps[:])
                    rot = pp.tile([D, TC1], F32, tag="rot", bufs=2)
                    nc.tensor.matmul(rot[:], psg_sb[:], praw[:],
                                     start=True, stop=True)
                    t1 = p1.tile([D, TC1], F32, tag="t1", bufs=2)
                    nc.vector.tensor_mul(t1[:], praw[:], cos_sb[:, tp:tp + TC1])
                    t2 = p1.tile([D, TC1], F32, tag="t2", bufs=2)
                    nc.vector.tensor_mul(t2[:], rot[:], sin_sb[:, tp:tp + TC1])
                    nc.vector.tensor_add(dst[h][:, ts:ts + TC1], t1[:], t2[:])
            # v projection: out [t, d] per 128-row t-tile
            for tt in range(TC1 // 128):
                jt = ts // 128 + tt
                ps = pp.tile([128, HPC * D], F32, tag="proj", bufs=4)
                for cc in range(KT):
                    nc.tensor.matmul(
                        ps[:], xt[:, cc, tt * 128:(tt + 1) * 128],
                        wv_sb[:, cc, :],
                        start=(cc == 0), stop=(cc == KT - 1))
                nc.vector.tensor_copy(vt[jt][:], ps[:])


def _phase2(nc, tc, plan, n_bias, bias_d, qT, kT, vt, ones_sb,
            a2a_in, a2a_out):
    with tc.tile_pool(name="p2t", bufs=1) as p2, \
         tc.tile_pool(name="ps2", bufs=1, space="PSUM") as pp:
        bias_sb = p2.tile([128, n_bias, TC2], F32)
        nc.sync.dma_start(bias_sb[:], bias_d.rearrange("u p m -> p u m"))
        for h in range(HPC):
            for b in range(B):
                for ci in range(CI):
                    gci = b * CI + ci      # global chunk == dest rank
                    live = [(jt, plan[ci][jt]) for jt in range(JT)
                            if plan[ci][jt] is not None]
                    if not live:
                        z = p2.tile([128, TC2], F32R, tag="ot", bufs=3)
                        nc.vector.memset(z[:], 0.0)
                        nc.sync.dma_start(a2a_in[h][gci, :, :], z[:])
                        continue
                    outp = pp.tile([D, TC2], F32, tag="outT", bufs=3)
                    rp = pp.tile([1, TC2], F32, tag="r", bufs=2)
                    i0 = b * T + ci * TC2
                    qs = qT[h][:, i0:i0 + TC2]
                    for idx, (jt, code) in enumerate(live):
                        jv = (b * T) // 128 + jt
                        sc = pp.tile([128, TC2], F32, tag="sc", bufs=3)
                        nc.tensor.matmul(
                            sc[:],
                            kT[h][:, b * T + jt * 128:b * T + (jt + 1) * 128],
                            qs, start=True, stop=True)
                        if code >= 0:
                            mt = p2.tile([128, TC2], F32, tag="mt", bufs=2)
                            nc.vector.tensor_add(mt[:], sc[:],
                                                 bias_sb[:, code, :])
                            src = mt
                        else:
                            src = sc
                        pt = p2.tile([128, TC2], F32R, tag="pt", bufs=4)
                        nc.scalar.activation(
                            pt[:], src[:], mybir.ActivationFunctionType.Exp,
                            bias=0.0, scale=float(SCALE))
                        nc.tensor.matmul(
                            outp[:], vt[jv][:, h * D:(h + 1) * D], pt[:],
                            start=(idx == 0), stop=(idx == len(live) - 1))
                        nc.tensor.matmul(
                            rp[:], ones_sb[:], pt[:],
                            start=(idx == 0), stop=(idx == len(live) - 1))
                    ri = p2.tile([1, TC2], F32, tag="ri", bufs=2)
                    nc.vector.reciprocal(ri[:], rp[:])
                    rb = p2.tile([128, TC2], F32, tag="rb", bufs=2)
                    nc.gpsimd.partition_broadcast(rb[:], ri[:])
                    ot = p2.tile([128, TC2], F32R, tag="ot", bufs=3)
                    nc.vector.tensor_mul(ot[:], outp[:], rb[:])
                    nc.sync.dma_start(a2a_in[h][gci, :, :], ot[:])
            # this head's comm overlaps the next head's compute
            nc.gpsimd.collective_compute(
                "AllToAll", mybir.AluOpType.bypass,
                replica_groups=[list(range(N_CORES))],
                ins=[a2a_in[h].opt()], outs=[a2a_out[h].opt()])


def _phase3(nc, tc, wop, wo_e, a2a_out, y):
    """Pass 1 computes the even hd-tiles (head 0 of each src rank -- data from
    the first AllToAll) into SBUF partials while the second AllToAll is still
    in flight; pass 2 computes odd tiles and adds the partials on evacuation,
    so the PE never blocks in-order on not-yet-arrived tiles."""
    HD2 = KT // 2
    with tc.tile_pool(name="ao", bufs=1) as aop, \
         tc.tile_pool(name="p3e", bufs=1) as p3e, \
         tc.tile_pool(name="ps3", bufs=1, space="PSUM") as pp:
        ao = []
        for hdt in range(N_HEADS * D // 128):
            s, k = divmod(hdt, HPC)
            t_ = aop.tile([128, TC2], F32R, name=f"ao{hdt}")
            nc.sync.dma_start(t_[:], a2a_out[k][s, :, :])
            ao.append(t_)
        ye = {}
        for par in range(2):
            for cj in range(C // TC2):
                wh = wop.tile([128, HD2, TC2], F32R, tag="wo")
                for q_ in range(4):
                    nc.sync.dma_start(wh[:, 2 * q_:2 * (q_ + 1), :],
                                      wo_e[:, par, 2 * q_:2 * (q_ + 1), cj, :])
                for tt in range(TSL // 128):
                    yp = pp.tile([128, TC2], F32, tag="y", bufs=4)
                    for n_ in range(HD2):
                        nc.tensor.matmul(
                            yp[:], ao[2 * n_ + par][:, tt * 128:(tt + 1) * 128],
                            wh[:, n_, :],
                            start=(n_ == 0), stop=(n_ == HD2 - 1))
                    if par == 0:
                        w_ = p3e.tile([128, TC2], F32, name=f"ye{cj}_{tt}")
                        nc.vector.tensor_copy(w_[:], yp[:])
                        ye[(cj, tt)] = w_
                    else:
                        ysb = p3e.tile([128, TC2], F32, tag="ysb", bufs=4)
                        nc.vector.tensor_add(ysb[:], yp[:], ye[(cj, tt)][:])
                        nc.sync.dma_start(
                            y[tt * 128:(tt + 1) * 128,
                              cj * TC2:(cj + 1) * TC2], ysb[:])


def _build(plan, n_bias):
    nc = bacc.Bacc("TRN2", num_devices=N_CORES)

    xT = nc.dram_tensor("xT", [C, BT], F32R, kind="ExternalInput")
    wq = nc.dram_tensor("wq", [C, HPC * D], F32R, kind="ExternalInput")
    wk = nc.dram_tensor("wk", [C, HPC * D], F32R, kind="ExternalInput")
    wv = nc.dram_tensor("wv", [C, HPC * D], F32R, kind="ExternalInput")
    wo = nc.dram_tensor("wo", [N_HEADS * D, C], F32R, kind="ExternalInput")
    cos_d = nc.dram_tensor("cos", [D, T], F32, kind="ExternalInput")
    sin_d = nc.dram_tensor("sin", [D, T], F32, kind="ExternalInput")
    psg_d = nc.dram_tensor("psg", [D, D], F32R, kind="ExternalInput")
    ones_d = nc.dram_tensor("ones", [128, 1], F32R, kind="ExternalInput")
    bias_d = nc.dram_tensor("bias", [n_bias, 128, TC2], F32, kind="ExternalInput")
    y = nc.dram_tensor("y", [TSL, C], F32, kind="ExternalOutput")

    xT_r = xT.rearrange("(n p) t -> p n t", p=128)
    wq_r = wq.rearrange("(n p) (h d) -> p n h d", p=128, d=D)
    wk_r = wk.rearrange("(n p) (h d) -> p n h d", p=128, d=D)
    wo_e = wo.rearrange("(n2 two p) (cb m) -> p two n2 cb m",
                        two=HPC, p=128, m=TC2)

    with tile.TileContext(nc) as tc:
        with tc.tile_pool(name="const", bufs=1) as cpool, \
             tc.tile_pool(name="dram", bufs=1, space="DRAM") as dram:

            a2a_in = [dram.tile([N_CORES, D, TC2], F32R, name=f"a2ai{h}")
                      for h in range(HPC)]
            a2a_out = [dram.tile([N_CORES, D, TC2], F32R, name=f"a2ao{h}")
                       for h in range(HPC)]

            with tc.tile_pool(name="qkv", bufs=1) as qkv:
                qT = [qkv.tile([D, BT], F32R, name=f"qT{h}") for h in range(HPC)]
                kT = [qkv.tile([D, BT], F32R, name=f"kT{h}") for h in range(HPC)]
                vt = [qkv.tile([128, HPC * D], F32R, name=f"v{j}")
                      for j in range(BT // 128)]

                with tc.tile_pool(name="wp", bufs=1) as wp:
                    # per-head weight tiles, split so the first matmul
                    # group's dependencies are small
                    wq_h = []
                    wk_h = []
                    for h in range(HPC):
                        w_ = wp.tile([128, KT, D], F32R, name=f"wqh{h}")
                        for q_ in range(4):
                            nc.sync.dma_start(
                                w_[:, 4 * q_:4 * (q_ + 1), :],
                                wq_r[:, 4 * q_:4 * (q_ + 1), h, :])
                        wq_h.append(w_)
                    psg_sb = cpool.tile([D, D], F32R)
                    nc.sync.dma_start(psg_sb[:], psg_d[:])
                    ones_sb = cpool.tile([128, 1], F32R)
                    nc.sync.dma_start(ones_sb[:], ones_d[:])
                    warm = cpool.tile([128, 1], F32)
                    nc.scalar.activation(warm[:], ones_sb[:],
                                         mybir.ActivationFunctionType.Exp,
                                         bias=0.0, scale=1.0)
                    warm2 = cpool.tile([128, 1], F32R)
                    nc.gpsimd.partition_broadcast(warm2[:], ones_sb[0:1, :])
                    for h in range(HPC):
                        w_ = wp.tile([128, KT, D], F32R, name=f"wkh{h}")
                        nc.sync.dma_start(w_[:, 0:KT // 2, :],
                                          wk_r[:, 0:KT // 2, h, :])
                        nc.sync.dma_start(w_[:, KT // 2:KT, :],
                                          wk_r[:, KT // 2:KT, h, :])
                        wk_h.append(w_)
                    cos_sb = wp.tile([D, T], F32)
                    nc.sync.dma_start(cos_sb[:], cos_d[:])
                    sin_sb = wp.tile([D, T], F32)
                    nc.sync.dma_start(sin_sb[:], sin_d[:])
                    wv_sb = wp.tile([128, KT, HPC * D], F32R)
                    nc.sync.dma_start(wv_sb[:],
                                      wv.rearrange("(n p) m -> p n m", p=128))

                    _phase1(nc, tc, (qT, kT, vt, wq_h, wk_h, wv_sb, psg_sb),
                            xT_r, cos_sb, sin_sb)

                # wo pool opens as soon as the phase-1 weights are freed so
                # the scheduler can hoist wo loads under phase-2 compute
                with tc.tile_pool(name="wo", bufs=2) as wop:
                    _phase2(nc, tc, plan, n_bias, bias_d, qT, kT, vt,
                            ones_sb, a2a_in, a2a_out)
                    _phase3(nc, tc, wop, wo_e, a2a_out, y)

    nc.finalize()
    return nc


_cache = {}


def _get_kernel(mask2d):
    key = mask2d.tobytes()
    if key not in _cache:
        plan, bias_tiles = _mask_plan(mask2d)
        nc = _build(plan, bias_tiles.shape[0])
        _cache[key] = (nc, bias_tiles)
    return _cache[key]


def kernel(x, mask, wq, wk, wv, wo, _trace=False):
    x = np.asarray(x)
    mask2d = np.asarray(mask).reshape(T, T).astype(bool)
    nc, bias_tiles = _get_kernel(mask2d)

    cosI, sinI, psignT = _rope_tables()
    xT_full = _round_f32r(np.asarray(x).reshape(BT, C).T)
    common = {
        "cos": cosI, "sin": sinI, "psg": psignT,
        "ones": np.ones((128, 1), np.float32),
        "bias": bias_tiles, "wo": _round_f32r(wo), "xT": xT_full,
    }
    in_maps = []
    for c in range(N_CORES):
        sl = slice(c * HPC * D, (c + 1) * HPC * D)
        in_maps.append({
            "wq": _round_f32r(np.asarray(wq)[:, sl]),
            "wk": _round_f32r(np.asarray(wk)[:, sl]),
            "wv": _round_f32r(np.asarray(wv)[:, sl]),
            **common,
        })

    r = run_bass_kernel_spmd(nc, in_maps, core_ids=list(range(N_CORES)),
                             trace=_trace)
    out = np.empty((BT, C), np.float32)
    for c in range(N_CORES):
        out[c * TSL:(c + 1) * TSL, :] = r.results[c]["y"]
    if _trace:
        kernel.last_results = r
    return out.reshape(B, T, C)


# revision 22
# speedup vs baseline: 1.1004x; 1.1004x over previous
"""Multi-head causal attention (RoPE) forward on 8 Trainium2 NeuronCores.

Sharding: tensor-parallel over heads -- 8 cores x 2 heads, each core handling
both batch elements (the flattened (B*T) = 4096 "time" axis).
Per core:
  phase 1: qT/kT [d, B*T] and v [B*T, d] projections from host-pre-transposed
           xT, RoPE applied via a +-1 pair-swap permutation matmul on PE plus
           elementwise combine with host-precomputed interleaved cos/sin.
  phase 2: per (head, batch), scores^T [j, i] = kT^T @ qT, exp on ScalarE (no
           max pass -- the score distribution is bounded and softmax is
           shift-invariant), mask as additive bias on partially-masked tiles
           only, fully-masked tiles skipped; out^T [d, i] and the softmax
           denominators (ones-matmul) accumulate on PE; normalized on evac.
           Each head's attention output leaves via its own 8-rank AllToAll
           (head-split -> t-split) so comm overlaps the next head's compute.
  phase 3: y[t-slice, :] = outT_full^T @ wo with wo streamed per half-chunk.
Host assembles the 8 t-slices into the full (B, T, C) output.

All matmuls run in float32r (fp32 with 12-bit mantissa, 1 col/cycle on the
PE): measured rel err ~2e-4 vs fp64 on 2048-deep dots.
"""

import os
import sys

import numpy as np

for _p in ("/opt/trn_rl_repo", "/root/.axon_site/_ro/trn_rl_repo"):
    if os.path.isdir(_p) and _p not in sys.path:
        sys.path.append(_p)

import concourse.bacc as bacc
import concourse.tile as tile
from concourse import mybir
from concourse.bass_utils import run_bass_kernel_spmd

B, T, C = 2, 2048, 2048
N_HEADS, D = 16, 128
THETA = 10000.0
N_CORES = 8
HPC = N_HEADS // N_CORES     # heads per core
BT = B * T                   # flattened time axis
TSL = BT // N_CORES          # per-core output slice after the all-to-all
KT = C // 128                # contraction chunks
TC1 = 256                    # phase-1 t-chunk (moving free dim)
NTC1 = BT // TC1
TC2 = 512                    # phase-2/3 chunk
CI = T // TC2                # i-chunks per (head, batch)
JT = T // 128                # j-tiles per (head, batch)
SCALE = 1.0 / np.sqrt(D)
MASKED_BIAS = -1.0e6         # pre-scale units; exp(SCALE*(s+bias)) == 0

F32R = mybir.dt.float32r
F32 = mybir.dt.float32


def _round_f32r(a):
    a = np.ascontiguousarray(a, dtype=np.float32)
    return (a.view(np.uint32) & np.uint32(0xFFFFF800)).view(np.float32)


def _mask_plan(mask2d):
    """Per (ci, jt) code: None=skip (all masked), -1=free (none masked),
    >=0 = index of partial-mask bias tile. scoresT tile (jt, ci) holds
    mask2d[i, j] transposed: bias[j_loc, i_loc] <- mask2d[TC2*ci+i, 128*jt+j].
    """
    uniq = {}
    tiles = []
    plan = []
    for ci in range(CI):
        row = []
        for jt in range(JT):
            blk = mask2d[TC2 * ci:TC2 * (ci + 1), 128 * jt:128 * (jt + 1)]
            if blk.all():
                row.append(-1)
            elif not blk.any():
                row.append(None)
            else:
                bias = np.where(blk.T, 0.0, np.float32(MASKED_BIAS)).astype(np.float32)
                key = bias.tobytes()
                if key not in uniq:
                    uniq[key] = len(tiles)
                    tiles.append(bias)
                row.append(uniq[key])
        plan.append(row)
    if not tiles:  # keep the DRAM tensor non-empty
        tiles.append(np.zeros((128, TC2), np.float32))
    return plan, np.stack(tiles)


def _rope_tables():
    inv_freq = 1.0 / (THETA ** (np.arange(0, D, 2, dtype=np.float64) / D))
    freqs = np.outer(inv_freq, np.arange(T, dtype=np.float64))  # [64, T]
    cosI = np.repeat(np.cos(freqs), 2, axis=0).astype(np.float32)  # [128, T]
    sinI = np.repeat(np.sin(freqs), 2, axis=0).astype(np.float32)
    # rot = psignT.T @ x : rot[2i] = -x[2i+1], rot[2i+1] = x[2i]
    psignT = np.zeros((D, D), np.float32)
    for i in range(D // 2):
        psignT[2 * i + 1, 2 * i] = -1.0
        psignT[2 * i, 2 * i + 1] = 1.0
    return cosI, sinI, psignT


def _phase1(nc, tc, qkv_tensors, xT_r, cos_sb, sin_sb):
    qT, kT, vt, wq_h, wk_h, wv_sb, psg_sb = qkv_tensors  # w/cos/sin live in wpool
    with tc.tile_pool(name="xt", bufs=2) as xp, \
         tc.tile_pool(name="p1t", bufs=1) as p1, \
         tc.tile_pool(name="ps1", bufs=1, space="PSUM") as pp:
        for tcn in range(NTC1):
            ts = tcn * TC1           # position in flattened BT
            tp = ts % T              # rope position (restarts per batch)
            xt = xp.tile([128, KT, TC1], F32R, tag="xt")
            nparts = 4 if tcn == 0 else 2
            step = KT // nparts
            for q_ in range(nparts):
                nc.sync.dma_start(xt[:, q_ * step:(q_ + 1) * step, :],
                                  xT_r[:, q_ * step:(q_ + 1) * step,
                                       ts:ts + TC1])
            for dst, w_h in ((qT, wq_h), (kT, wk_h)):
                for h in range(HPC):
                    ps = pp.tile([D, TC1], F32, tag="proj", bufs=4)
                    for cc in range(KT):
                        nc.tensor.matmul(
                            ps[:], w_h[h][:, cc, :], xt[:, cc, :],
                            start=(cc == 0), stop=(cc == KT - 1))
                    praw = p1.tile([D, TC1], F32R, tag="praw", bufs=3)
                    nc.vector.tensor_copy(praw[:], ps[:])
                    rot = pp.tile([D, TC1], F32, tag="rot", bufs=2)
                    nc.tensor.matmul(rot[:], psg_sb[:], praw[:],
                                     start=True, stop=True)
                    t1 = p1.tile([D, TC1], F32, tag="t1", bufs=2)
                    nc.vector.tensor_mul(t1[:], praw[:], cos_sb[:, tp:tp + TC1])
                    t2 = p1.tile([D, TC1], F32, tag="t2", bufs=2)
                    nc.vector.tensor_mul(t2[:], rot[:], sin_sb[:, tp:tp + TC1])
                    nc.vector.tensor_add(dst[h][:, ts:ts + TC1], t1[:], t2[:])
            # v projection: out [t, d] per 128-row t-tile
            for tt in range(TC1 // 128):
                jt = ts // 128 + tt
                ps = pp.tile([128, HPC * D], F32, tag="proj", bufs=4)
                for cc in range(KT):
                    nc.tensor.matmul(
                        ps[:], xt[:, cc, tt * 128:(tt + 1) * 128],
                        wv_sb[:, cc, :],
                        start=(cc == 0), stop=(cc == KT - 1))
                nc.vector.tensor_copy(vt[jt][:], ps[:])


def _phase2(nc, tc, plan, n_bias, bias_d, qT, kT, vt, ones_sb,
            a2a_in, a2a_out):
    with tc.tile_pool(name="p2t", bufs=1) as p2, \
         tc.tile_pool(name="ps2", bufs=1, space="PSUM") as pp:
        bias_sb = p2.tile([128, n_bias, TC2], F32)
        nc.sync.dma_start(bias_sb[:], bias_d.rearrange("u p m -> p u m"))
        for h in range(HPC):
            for b in range(B):
                for ci in range(CI):
                    gci = b * CI + ci      # global chunk == dest rank
                    live = [(jt, plan[ci][jt]) for jt in range(JT)
                            if plan[ci][jt] is not None]
                    if not live:
                        z = p2.tile([128, TC2], F32R, tag="ot", bufs=3)
                        nc.vector.memset(z[:], 0.0)
                        nc.sync.dma_start(a2a_in[h][gci, :, :], z[:])
                        continue
                    outp = pp.tile([D, TC2], F32, tag="outT", bufs=3)
                    rp = pp.tile([1, TC2], F32, tag="r", bufs=2)
                    i0 = b * T + ci * TC2
                    qs = qT[h][:, i0:i0 + TC2]
                    for idx, (jt, code) in enumerate(live):
                        jv = (b * T) // 128 + jt
                        sc = pp.tile([128, TC2], F32, tag="sc", bufs=3)
                        nc.tensor.matmul(
                            sc[:],
                            kT[h][:, b * T + jt * 128:b * T + (jt + 1) * 128],
                            qs, start=True, stop=True)
                        if code >= 0:
                            mt = p2.tile([128, TC2], F32, tag="mt", bufs=2)
                            nc.vector.tensor_add(mt[:], sc[:],
                                                 bias_sb[:, code, :])
                            src = mt
                        else:
                            src = sc
                        pt = p2.tile([128, TC2], F32R, tag="pt", bufs=4)
                        nc.scalar.activation(
                            pt[:], src[:], mybir.ActivationFunctionType.Exp,
                            bias=0.0, scale=float(SCALE))
                        nc.tensor.matmul(
                            outp[:], vt[jv][:, h * D:(h + 1) * D], pt[:],
                            start=(idx == 0), stop=(idx == len(live) - 1))
                        nc.tensor.matmul(
                            rp[:], ones_sb[:], pt[:],
                            start=(idx == 0), stop=(idx == len(live) - 1))
                    ri = p2.tile([1, TC2], F32, tag="ri", bufs=2)
                    nc.vector.reciprocal(ri[:], rp[:])
                    rb = p2.tile([128, TC2], F32, tag="rb", bufs=2)
                    nc.gpsimd.partition_broadcast(rb[:], ri[:])
                    ot = p2.tile([128, TC2], F32R, tag="ot", bufs=3)
                    nc.vector.tensor_mul(ot[:], outp[:], rb[:])
                    nc.sync.dma_start(a2a_in[h][gci, :, :], ot[:])
            # this head's comm overlaps the next head's compute
            nc.gpsimd.collective_compute(
                "AllToAll", mybir.AluOpType.bypass,
                replica_groups=[list(range(N_CORES))],
                ins=[a2a_in[h].opt()], outs=[a2a_out[h].opt()])


def _phase3(nc, tc, wop, wo_r, a2a_out, y):
    HD2 = KT // 2
    with tc.tile_pool(name="ao", bufs=1) as aop, \
         tc.tile_pool(name="ps3", bufs=1, space="PSUM") as pp:
        ao = []
        for hdt in range(N_HEADS * D // 128):
            s, k = divmod(hdt, HPC)
            t_ = aop.tile([128, TC2], F32R, name=f"ao{hdt}")
            nc.sync.dma_start(t_[:], a2a_out[k][s, :, :])
            ao.append(t_)
        for cj in range(C // TC2):
            wha = wop.tile([128, HD2, TC2], F32R, tag="woa")
            for q_ in range(4):
                nc.sync.dma_start(wha[:, 2 * q_:2 * (q_ + 1), :],
                                  wo_r[:, 2 * q_:2 * (q_ + 1), cj, :])
            whb = wop.tile([128, HD2, TC2], F32R, tag="wob")
            for q_ in range(4):
                nc.sync.dma_start(whb[:, 2 * q_:2 * (q_ + 1), :],
                                  wo_r[:, HD2 + 2 * q_:HD2 + 2 * (q_ + 1), cj, :])
            for tt in range(TSL // 128):
                yp = pp.tile([128, TC2], F32, tag="y", bufs=4)
                order = [i for i in range(KT) if i % HPC == 0] + \
                        [i for i in range(KT) if i % HPC != 0]
                for n_, hdt in enumerate(order):
                    w_t = wha[:, hdt, :] if hdt < HD2 else whb[:, hdt - HD2, :]
                    nc.tensor.matmul(
                        yp[:], ao[hdt][:, tt * 128:(tt + 1) * 128], w_t,
                        start=(n_ == 0), stop=(n_ == KT - 1))
                ysb = wop.tile([128, TC2], F32, tag="ysb", bufs=3)
                nc.vector.tensor_copy(ysb[:], yp[:])
                nc.sync.dma_start(
                    y[tt * 128:(tt + 1) * 128, cj * TC2:(cj + 1) * TC2],
                    ysb[:])


def _build(plan, n_bias):
    nc = bacc.Bacc("TRN2", num_devices=N_CORES)

    xT = nc.dram_tensor("xT", [C, BT], F32R, kind="ExternalInput")
    wq = nc.dram_tensor("wq", [C, HPC * D], F32R, kind="ExternalInput")
    wk = nc.dram_tensor("wk", [C, HPC * D], F32R, kind="ExternalInput")
    wv = nc.dram_tensor("wv", [C, HPC * D], F32R, kind="ExternalInput")
    wo = nc.dram_tensor("wo", [N_HEADS * D, C], F32R, kind="ExternalInput")
    cos_d = nc.dram_tensor("cos", [D, T], F32, kind="ExternalInput")
    sin_d = nc.dram_tensor("sin", [D, T], F32, kind="ExternalInput")
    psg_d = nc.dram_tensor("psg", [D, D], F32R, kind="ExternalInput")
    ones_d = nc.dram_tensor("ones", [128, 1], F32R, kind="ExternalInput")
    bias_d = nc.dram_tensor("bias", [n_bias, 128, TC2], F32, kind="ExternalInput")
    y = nc.dram_tensor("y", [TSL, C], F32, kind="ExternalOutput")

    xT_r = xT.rearrange("(n p) t -> p n t", p=128)
    wq_r = wq.rearrange("(n p) (h d) -> p n h d", p=128, d=D)
    wk_r = wk.rearrange("(n p) (h d) -> p n h d", p=128, d=D)
    wo_r = wo.rearrange("(n p) (cb m) -> p n cb m", p=128, m=TC2)

    with tile.TileContext(nc) as tc:
        with tc.tile_pool(name="const", bufs=1) as cpool, \
             tc.tile_pool(name="dram", bufs=1, space="DRAM") as dram:

            a2a_in = [dram.tile([N_CORES, D, TC2], F32R, name=f"a2ai{h}")
                      for h in range(HPC)]
            a2a_out = [dram.tile([N_CORES, D, TC2], F32R, name=f"a2ao{h}")
                       for h in range(HPC)]

            with tc.tile_pool(name="qkv", bufs=1) as qkv:
                qT = [qkv.tile([D, BT], F32R, name=f"qT{h}") for h in range(HPC)]
                kT = [qkv.tile([D, BT], F32R, name=f"kT{h}") for h in range(HPC)]
                vt = [qkv.tile([128, HPC * D], F32R, name=f"v{j}")
                      for j in range(BT // 128)]

                with tc.tile_pool(name="wp", bufs=1) as wp:
                    # per-head weight tiles, split so the first matmul
                    # group's dependencies are small
                    wq_h = []
                    wk_h = []
                    for h in range(HPC):
                        w_ = wp.tile([128, KT, D], F32R, name=f"wqh{h}")
                        for q_ in range(4):
                            nc.sync.dma_start(
                                w_[:, 4 * q_:4 * (q_ + 1), :],
                                wq_r[:, 4 * q_:4 * (q_ + 1), h, :])
                        wq_h.append(w_)
                    psg_sb = cpool.tile([D, D], F32R)
                    nc.sync.dma_start(psg_sb[:], psg_d[:])
                    ones_sb = cpool.tile([128, 1], F32R)
                    nc.sync.dma_start(ones_sb[:], ones_d[:])
                    warm = cpool.tile([128, 1], F32)
                    nc.scalar.activation(warm[:], ones_sb[:],
                                         mybir.ActivationFunctionType.Exp,
                                         bias=0.0, scale=1.0)
                    warm2 = cpool.tile([128, 1], F32R)
                    nc.gpsimd.partition_broadcast(warm2[:], ones_sb[0:1, :])
                    for h in range(HPC):
                        w_ = wp.tile([128, KT, D], F32R, name=f"wkh{h}")
                        nc.sync.dma_start(w_[:, 0:KT // 2, :],
                                          wk_r[:, 0:KT // 2, h, :])
                        nc.sync.dma_start(w_[:, KT // 2:KT, :],
                                          wk_r[:, KT // 2:KT, h, :])
                        wk_h.append(w_)
                    cos_sb = wp.tile([D, T], F32)
                    nc.sync.dma_start(cos_sb[:], cos_d[:])
                    sin_sb = wp.tile([D, T], F32)
                    nc.sync.dma_start(sin_sb[:], sin_d[:])
                    wv_sb = wp.tile([128, KT, HPC * D], F32R)
                    nc.sync.dma_start(wv_sb[:],
                                      wv.rearrange("(n p) m -> p n m", p=128))

                    _phase1(nc, tc, (qT, kT, vt, wq_h, wk_h, wv_sb, psg_sb),
                            xT_r, cos_sb, sin_sb)

                # wo pool opens as soon as the phase-1 weights are freed so
                # the scheduler can hoist wo loads under phase-2 compute
                with tc.tile_pool(name="wo", bufs=2) as wop:
                    _phase2(nc, tc, plan, n_bias, bias_d, qT, kT, vt,
                            ones_sb, a2a_in, a2a_out)
                    _phase3(nc, tc, wop, wo_r, a2a_out, y)

    nc.finalize()
    return nc


_cache = {}


def _get_kernel(mask2d):
    key = mask2d.tobytes()
    if key not in _cache:
        plan, bias_tiles = _mask_plan(mask2d)
        nc = _build(plan, bias_tiles.shape[0])
        _cache[key] = (nc, bias_tiles)
    return _cache[key]


def kernel(x, mask, wq, wk, wv, wo, _trace=False):
    x = np.asarray(x)
    mask2d = np.asarray(mask).reshape(T, T).astype(bool)
    nc, bias_tiles = _get_kernel(mask2d)

    cosI, sinI, psignT = _rope_tables()
    xT_full = _round_f32r(np.asarray(x).reshape(BT, C).T)
    common = {
        "cos": cosI, "sin": sinI, "psg": psignT,
        "ones": np.ones((128, 1), np.float32),
        "bias": bias_tiles, "wo": _round_f32r(wo), "xT": xT_full,
    }
    in_maps = []
    for c in range(N_CORES):
        sl = slice(c * HPC * D, (c + 1) * HPC * D)
        in_maps.append({
            "wq": _round_f32r(np.asarray(wq)[:, sl]),
            "wk": _round_f32r(np.asarray(wk)[:, sl]),
            "wv": _round_f32r(np.asarray(wv)[:, sl]),
            **common,
        })

    r = run_bass_kernel_spmd(nc, in_maps, core_ids=list(range(N_CORES)),
                             trace=_trace)
    out = np.empty((BT, C), np.float32)
    for c in range(N_CORES):
        out[c * TSL:(c + 1) * TSL, :] = r.results[c]["y"]
    if _trace:
        kernel.last_results = r
    return out.reshape(B, T, C)
